# revision 24
# baseline (speedup 1.0000x reference)
"""DeepseekV3 MoE layer on 8 trn2 NeuronCores (expert-parallel).

Strategy
--------
* Routing (sigmoid gate + grouped top-k) runs on host in numpy: it is
  ~0.1% of the FLOPs and it *determines* the sharding (which tokens go
  to which core), i.e. it is the dispatch step of the all-to-all.
* Experts are sharded 4-per-core, assigned by sorted load so that slot
  capacities (compile-time matmul shapes, shared by all cores under
  SPMD) can be tight: slot s on every core gets an expert from
  load-rank group s, and the slot capacity is the rank-group max.
* Gate/up expert weights are streamed as fp8 e3m4 (x64 scale, undone
  by the Silu activation's scale argument): the kernel is HBM-bound on
  weight streaming, and the PE consumes fp8 stationary x bf16 moving
  at full per-element precision (fp22 upconvert, no DoubleRow).
  w_down and the shared expert stay bf16 for error margin.
* Per core, per expert slot: gT/uT = W@xT accumulated over 16 H-chunks
  in PSUM, a = silu(g)*u evicted to SBUF as bf16 [I, C]; the down
  projection runs transposed: y[h, t] accumulated over 11 I-chunks
  with N = cap token columns (no padded 512-wide tiles), DMA'd out as
  [H, C] bf16.  The combine weight (and the 1/64 fp8 scale) is applied
  on the host during scatter-add, which is free.
* The shared expert is sharded over its intermediate dim SI (352/core,
  padded to 384): every core computes a partial [T, H] contribution.
  Its gate/up half runs FIRST (small inputs -> PE busy while the big
  routed weight stream ramps) with per-m-tile weight DMAs so the first
  matmul starts after ~3 MB; its down-projection runs LAST.  Output
  stores go through SWDGE (gpsimd) so they never block load issue on
  the SP HWDGE ring.
* Host combine: sum the 8 shared partials, weighted scatter-add of the
  32 compact expert outputs.
"""

import numpy as np
import ml_dtypes

import concourse.bass as bass
import concourse.mybir as mybir
import concourse.tile as tile
from concourse.bass_utils import run_bass_kernel_spmd

BF16 = ml_dtypes.bfloat16
F8E3 = ml_dtypes.float8_e3m4
WSCALE = 64.0

# ---- problem constants (fixed by the spec) ----
E, G, EPG, TKG, TOPK = 32, 8, 4, 4, 4
H, I, SI, SCALE = 2048, 1408, 2816, 2.5
T = 1024
NCORES = 8
EPC = E // NCORES          # experts per core = 4
KH = H // 128              # 16 contraction chunks over H
MI = I // 128              # 11 tiles over I
NH = H // 128              # 16 output tiles over H (down projection)
SIL = SI // NCORES         # 352 local shared-intermediate
SIP = 384                  # padded to 3*128
KSI = SIP // 128           # 3
HT = H // 512              # 4 output tiles over H (shared down)
TT = T // 512              # 2 tiles over tokens (shared gate/up)

_STATE: dict = {}

_TPB_ENGINES = {"Pool", "Activation", "PE", "DVE", "SP"}


def _split_multiwait_bir(bir_bytes: bytes) -> bytes:
    """Walrus codegen here accepts at most one sem-wait per TPB
    instruction.  Move excess waits onto single-wait NoOps inserted
    immediately before the instruction on the same engine (engine
    streams are in-order, and sem-ge waits are monotonic, so the chain
    is equivalent to the conjunction)."""
    import orjson

    bir = orjson.loads(bir_bytes)
    ctr = 0
    for f in bir["functions"]:
        for blk in f["blocks"]:
            out = []
            for inst in blk["instructions"]:
                si = inst.get("sync_info")
                waits = (si or {}).get("on_wait") or []
                if len(waits) > 1 and inst.get("engine") in _TPB_ENGINES:
                    for w in waits[:-1]:
                        ctr += 1
                        out.append({
                            "debug": inst.get("debug", 0),
                            "engine": inst["engine"],
                            "ins": [],
                            "outs": [],
                            "name": f"I-wsplit-{ctr}",
                            "opcode": "NoOp",
                            "sync_info": {"on_update": [], "on_wait": [w]},
                        })
                    si["on_wait"] = waits[-1:]
                out.append(inst)
            blk["instructions"] = out
    return orjson.dumps(bir)


def _patch_tile():
    if _STATE.get("patched"):
        return
    from concourse.tile import ScopedClock, TileContext

    _orig_to_json = bass.Bass.to_json_bytes

    def to_json_bytes_split(self):
        return _split_multiwait_bir(_orig_to_json(self))

    bass.Bass.to_json_bytes = to_json_bytes_split

    def _drain_and_barrier_split(self, tick_clock, wait_clock):
        probe = self.nc.sync.nop(nofuse=True)
        wait_clock.add_sem_waits(
            probe.ins, ScopedClock({None: tick_clock.global_clock})
        )
        waits = list(probe.ins.sync_info.on_wait) if probe.ins.sync_info else []
        if probe.ins.sync_info:
            probe.ins.sync_info.on_wait = waits[:1]
            for w in waits[1:]:
                n2 = self.nc.sync.nop(nofuse=True)
                si = n2.ins.sync_info
                if si is None:
                    n2.ins.sync_info = mybir.SyncInfo(on_wait=[w], on_update=[])
                else:
                    si.on_wait = [w]
        self.nc.sync.drain()
        self.nc.all_engine_barrier()
        assert self.sems is not None
        popped = self.nc._tile_sem_poison_stack.pop()
        assert popped is self._sem_poison
        self.nc.clear_and_free_semaphores(list(self.sems.allocated().values()))
        self.nc.all_engine_barrier()

    TileContext._drain_and_barrier = _drain_and_barrier_split
    _STATE["patched"] = True


def _round_bf16(a: np.ndarray) -> np.ndarray:
    """fp32 -> bf16 with round-to-nearest-even, fast pure-numpy path."""
    u = np.ascontiguousarray(a, dtype=np.float32).view(np.uint32)
    r = ((u >> 16) & 1) + np.uint32(0x7FFF)
    return ((u + r) >> np.uint32(16)).astype(np.uint16).view(BF16)


# --------------------------------------------------------------------
# host routing — exact numpy mirror of the reference gate
# --------------------------------------------------------------------
def _gate_host(x, gate_weight, bias):
    Tn = x.shape[0]
    logits = x @ gate_weight.T                       # [T, E]
    scores = 1.0 / (1.0 + np.exp(-logits))
    sfc = scores + bias[None, :]
    gs = sfc.reshape(Tn, G, EPG)
    top2 = np.sort(gs, axis=-1)[:, :, -2:].sum(-1)   # [T, G]
    grp_idx = np.argsort(-top2, axis=-1, kind="stable")[:, :TKG]
    gmask = np.zeros((Tn, G), bool)
    gmask[np.arange(Tn)[:, None], grp_idx] = True
    smask = np.repeat(gmask, EPG, axis=1)
    tmp = np.where(smask, sfc, 0.0)
    topk_idx = np.argsort(-tmp, axis=-1, kind="stable")[:, :TOPK]
    topk_w = np.take_along_axis(scores, topk_idx, axis=1)
    topk_w = topk_w / (topk_w.sum(-1, keepdims=True) + 1e-20)
    return topk_idx, topk_w * SCALE


# --------------------------------------------------------------------
# device kernel (parameterized by per-slot capacities)
# --------------------------------------------------------------------
def _build_nc(caps):
    _patch_tile()
    nc = bass.Bass("TRN2", target_bir_lowering=False, debug=False, num_devices=1)
    f32, bf, f8 = mybir.dt.float32, mybir.dt.bfloat16, mybir.dt.float8e3
    CT = sum(caps)           # total token capacity per core
    CMX = max(caps)
    coff = [sum(caps[:s]) for s in range(EPC)]  # xg column offsets

    xg = nc.dram_tensor("xg", [128, KH * CT], bf, kind="ExternalInput").ap()
    wg = nc.dram_tensor("wg", [EPC, MI, 128, KH * 128], bf, kind="ExternalInput").ap()
    wu = nc.dram_tensor("wu", [EPC, MI, 128, KH * 128], f8, kind="ExternalInput").ap()
    wd = nc.dram_tensor("wd", [EPC, MI, 2, 128, H // 2], f8,
                        kind="ExternalInput").ap()
    xs = nc.dram_tensor("xs", [TT, 128, KH * 512], bf, kind="ExternalInput").ap()
    sg = nc.dram_tensor("sg", [KSI, 128, KH * 128], bf, kind="ExternalInput").ap()
    su = nc.dram_tensor("su", [KSI, 128, KH * 128], bf, kind="ExternalInput").ap()
    sd = nc.dram_tensor("sd", [128, KSI * H], bf, kind="ExternalInput").ap()
    yr = nc.dram_tensor("yr", [H, CT], bf, kind="ExternalOutput").ap()
    ys = nc.dram_tensor("ys", [T, H], bf, kind="ExternalOutput").ap()

    SILU = mybir.ActivationFunctionType.Silu

    with tile.TileContext(nc) as tc:
        with tc.tile_pool(name="main", bufs=1) as pool, \
             tc.tile_pool(name="psum", bufs=1, space="PSUM") as pp:
            # ---- startup DMAs, split across both HWDGE queues; the
            # m=0 shared tiles and the nt=0 x block are split in k-halves
            # so the first matmul starts after ~1.3 MB.  SP queue:
            # sg0/su0, xg, sd, the wu/wd fp8 streams and the tail ys
            # stores.  ACT queue: xs, sg1/2, su1/2 and the bf16 wg
            # stream.
            KH2 = KH // 2
            sg0h, su0h, xs0h = [], [], []
            for h in range(2):
                sg0h.append(pool.tile([128, KH2 * 128], bf, tag="sg0",
                                      bufs=2, name=f"sg0{h}"))
                su0h.append(pool.tile([128, KH2 * 128], bf, tag="su0",
                                      bufs=2, name=f"su0{h}"))
                xs0h.append(pool.tile([128, KH2 * 512], bf, tag="xs0",
                                      bufs=2, name=f"xs0{h}"))
            sg_sbs, su_sbs = [None], [None]
            for m in range(1, KSI):
                sg_sbs.append(pool.tile([128, KH * 128], bf, tag="sg",
                                        bufs=KSI - 1, name=f"sg{m}"))
                su_sbs.append(pool.tile([128, KH * 128], bf, tag="su",
                                        bufs=KSI - 1, name=f"su{m}"))
            xs1_sb = pool.tile([128, KH * 512], bf, tag="xs", bufs=1)
            xg_sb = pool.tile([128, KH * CT], bf, tag="xg", bufs=1)
            sd_sb = pool.tile([128, KSI * H], bf, tag="sd", bufs=1)
            # x blocks and sd ride the SWDGE (gpsimd) as a third DMA
            # stream: it starts ~7us before the HWDGE preamble finishes
            # and its bytes come off the two HW queues.
            nc.gpsimd.dma_start(xs0h[0][:], xs[0, :, :KH2 * 512])
            nc.gpsimd.dma_start(xs0h[1][:], xs[0, :, KH2 * 512:])
            nc.gpsimd.dma_start(xs1_sb[:], xs[1])
            nc.gpsimd.dma_start(sd_sb[:], sd[:])
            nc.sync.dma_start(sg0h[0][:], sg[0, :, :KH2 * 128])
            nc.sync.dma_start(su0h[0][:], su[0, :, :KH2 * 128])
            nc.sync.dma_start(sg0h[1][:], sg[0, :, KH2 * 128:])
            nc.sync.dma_start(su0h[1][:], su[0, :, KH2 * 128:])
            for m in range(1, KSI):
                nc.sync.dma_start(sg_sbs[m][:], sg[m])
                nc.sync.dma_start(su_sbs[m][:], su[m])
            nc.sync.dma_start(xg_sb[:], xg[:])

            def shared_w(ws, m, k):
                # weight lhsT [128, 128] for shared m-tile, chunk k
                if m == 0:
                    hh = sg0h if ws == "g" else su0h
                    t = hh[k // KH2]
                    kk = k % KH2
                else:
                    t = sg_sbs[m] if ws == "g" else su_sbs[m]
                    kk = k
                return t[:, kk * 128:(kk + 1) * 128]

            def shared_x(nt, k):
                if nt == 0:
                    return xs0h[k // KH2][:, (k % KH2) * 512:
                                          (k % KH2 + 1) * 512]
                return xs1_sb[:, k * 512:(k + 1) * 512]

            # ---- shared expert gate/up (sharded over SI) ----
            # nt-outer: the nt=0 groups need only xs0, deferring xs1's
            # 2.1 MB by ~20us so the front stream stays sustainable.
            as_sb = pool.tile([128, KSI * T], bf, tag="as", bufs=1)
            for nt in range(TT):
                for m in range(KSI):
                    pg = pp.tile([128, 512], f32, tag="pg", bufs=2,
                                 name=f"psg{m}_{nt}")
                    pu = pp.tile([128, 512], f32, tag="pu", bufs=2,
                                 name=f"psu{m}_{nt}")
                    for k in range(KH):
                        nc.tensor.matmul(
                            pg[:], shared_w("g", m, k), shared_x(nt, k),
                            start=(k == 0), stop=(k == KH - 1))
                    for k in range(KH):
                        nc.tensor.matmul(
                            pu[:], shared_w("u", m, k), shared_x(nt, k),
                            start=(k == 0), stop=(k == KH - 1))
                    sil = pool.tile([128, 512], f32, tag="sil", bufs=2,
                                    name=f"ssil{m}_{nt}")
                    nc.scalar.activation(sil[:], pg[:], SILU)
                    nc.vector.tensor_mul(
                        as_sb[:, m * T + nt * 512: m * T + (nt + 1) * 512],
                        sil[:], pu[:])

            # ---- routed experts ----
            for s in range(EPC):
                cap = caps[s]
                a_sb = pool.tile([128, MI * CMX], bf, tag="a", bufs=2,
                                 name=f"a{s}")
                for m in range(MI):
                    wg_sb = pool.tile([128, KH * 128], bf, tag="wg", bufs=8,
                                      name=f"wg{s}_{m}")
                    nc.scalar.dma_start(wg_sb[:], wg[s, m])
                    wu_sb = pool.tile([128, KH * 128], f8, tag="wu", bufs=8,
                                      name=f"wu{s}_{m}")
                    nc.sync.dma_start(wu_sb[:], wu[s, m])
                    pg = pp.tile([128, cap], f32, tag="pg", bufs=2,
                                 padded_shape=[128, 512], name=f"pg{s}_{m}")
                    pu = pp.tile([128, cap], f32, tag="pu", bufs=2,
                                 padded_shape=[128, 512], name=f"pu{s}_{m}")
                    for k in range(KH):
                        nc.tensor.matmul(
                            pg[:], wg_sb[:, k * 128:(k + 1) * 128],
                            xg_sb[:, k * CT + coff[s]: k * CT + coff[s] + cap],
                            start=(k == 0), stop=(k == KH - 1))
                    for k in range(KH):
                        nc.tensor.matmul(
                            pu[:], wu_sb[:, k * 128:(k + 1) * 128],
                            xg_sb[:, k * CT + coff[s]: k * CT + coff[s] + cap],
                            start=(k == 0), stop=(k == KH - 1))
                    sil = pool.tile([128, cap], f32, tag="sil", bufs=2,
                                    padded_shape=[128, 512], name=f"sil{s}_{m}")
                    nc.scalar.activation(sil[:], pg[:], SILU)
                    nc.vector.tensor_mul(
                        a_sb[:, m * cap:(m + 1) * cap], sil[:], pu[:])

                # w_down in half-tiles (A: H columns 0..1023, B: rest) so
                # the A halves free mid-way through the down phase and the
                # next slot's stream starts that much earlier.
                wda_sbs, wdb_sbs = [], []
                for k2 in range(MI):
                    wda = pool.tile([128, H // 2], f8, tag="wda", bufs=13,
                                    name=f"wda{s}_{k2}")
                    nc.sync.dma_start(wda[:], wd[s, k2, 0])
                    wda_sbs.append(wda)
                for k2 in range(MI):
                    wdb = pool.tile([128, H // 2], f8, tag="wdb", bufs=13,
                                    name=f"wdb{s}_{k2}")
                    nc.sync.dma_start(wdb[:], wd[s, k2, 1])
                    wdb_sbs.append(wdb)

                if s == EPC - 1:
                    # ---- shared expert down-projection, emitted between
                    # the last slot's gate/up and down phases: its ys
                    # stores are issued after the wd DMAs on the SP queue
                    # (no head-of-line block of loads) and drain while
                    # the last down phase computes, instead of as an
                    # exposed tail.
                    for mt in range(T // 128):
                        for n in range(HT):
                            py = pp.tile([128, 512], f32, tag="py", bufs=4,
                                         name=f"pys{mt}_{n}")
                            for k in range(KSI):
                                nc.tensor.matmul(
                                    py[:],
                                    as_sb[:, k * T + mt * 128:
                                          k * T + (mt + 1) * 128],
                                    sd_sb[:, k * H + n * 512:
                                          k * H + (n + 1) * 512],
                                    start=(k == 0), stop=(k == KSI - 1))
                            yo = pool.tile([128, 512], bf, tag="yo", bufs=8,
                                           name=f"yos{mt}_{n}")
                            nc.vector.tensor_copy(yo[:], py[:])
                            nc.sync.dma_start(
                                ys[mt * 128:(mt + 1) * 128,
                                   n * 512:(n + 1) * 512],
                                yo[:])

                # transposed down-projection: y[h, t], N = cap columns,
                # contraction over I.  No padded token tiles.
                for n in range(NH):
                    half = wda_sbs if n < NH // 2 else wdb_sbs
                    nn = n if n < NH // 2 else n - NH // 2
                    py = pp.tile([128, cap], f32, tag="py", bufs=4,
                                 padded_shape=[128, 512], name=f"py{s}_{n}")
                    for k2 in range(MI):
                        nc.tensor.matmul(
                            py[:],
                            half[k2][:, nn * 128:(nn + 1) * 128],
                            a_sb[:, k2 * cap: k2 * cap + cap],
                            start=(k2 == 0), stop=(k2 == MI - 1))
                    yo = pool.tile([128, cap], bf, tag="yo", bufs=8,
                                   padded_shape=[128, 512], name=f"yo{s}_{n}")
                    nc.vector.tensor_copy(yo[:], py[:])
                    if s == EPC - 1:
                        # last slot's stores drain through the ACT HWDGE
                        # queue (its wg load stream is done by now) so the
                        # kernel tail is not paced by SWDGE.
                        nc.scalar.dma_start(
                            yr[n * 128:(n + 1) * 128,
                               coff[s]: coff[s] + cap], yo[:])
                    else:
                        nc.gpsimd.dma_start(
                            yr[n * 128:(n + 1) * 128,
                               coff[s]: coff[s] + cap], yo[:])


    return nc


def _get_nc(caps):
    key = ("nc", tuple(caps))
    if key not in _STATE:
        _STATE[key] = _build_nc(caps)
    return _STATE[key]


# --------------------------------------------------------------------
# host packing
# --------------------------------------------------------------------
def _pack_weight_gate_up(w_e, dt, scale):
    # w_e: [I, H] f32 -> dt x scale, packed [MI, 128, KH*128]
    # with [m, p, k*128+c] = w[m*128+c, k*128+p]
    wq = (w_e * scale).astype(dt) if dt is F8E3 else _round_bf16(w_e)
    return np.ascontiguousarray(
        wq.reshape(MI, 128, KH, 128).transpose(0, 3, 2, 1)
    ).reshape(MI, 128, KH * 128)


def _pack_weight_down(w_e):
    # w_e: [H, I] f32 -> fp8 e3m4 x WSCALE, [MI, 2, 128, H/2] with
    # [k2, h2, p, hh] = w[h2*(H/2)+hh, k2*128+p]
    w8 = (w_e * WSCALE).astype(F8E3)
    return np.ascontiguousarray(
        w8.reshape(2, H // 2, MI, 128).transpose(2, 0, 3, 1))


def _pack_hchunks(a16):
    # a16: [H, N] bf16 -> [128, KH*N] with [p, k*N+j] = a[k*128+p, j]
    N = a16.shape[1]
    return np.ascontiguousarray(
        a16.reshape(KH, 128, N).transpose(1, 0, 2)).reshape(128, KH * N)


def _weight_packs(inp):
    """Pack (and cache) the routed + shared weights; they do not depend
    on routing, only on the weight tensors themselves."""
    key = tuple(inp[k].ctypes.data for k in
                ("w_gate", "w_up", "w_down", "shared_w_gate",
                 "shared_w_up", "shared_w_down"))
    cached = _STATE.get("wpack")
    if cached is not None and cached[0] == key:
        return cached[1]

    packs = {
        "wg": [_pack_weight_gate_up(inp["w_gate"][e], BF16, 1.0)
               for e in range(E)],
        "wu": [_pack_weight_gate_up(inp["w_up"][e], F8E3, WSCALE)
               for e in range(E)],
        "wd": [_pack_weight_down(inp["w_down"][e]) for e in range(E)],
    }
    sgT = _round_bf16(inp["shared_w_gate"]).T        # [H, SI]
    suT = _round_bf16(inp["shared_w_up"]).T
    sdT = _round_bf16(inp["shared_w_down"]).T        # [SI, H]
    sg_l, su_l, sd_l = [], [], []
    for c in range(NCORES):
        sg_pad = np.zeros((H, SIP), BF16)
        sg_pad[:, :SIL] = sgT[:, c * SIL:(c + 1) * SIL]
        su_pad = np.zeros((H, SIP), BF16)
        su_pad[:, :SIL] = suT[:, c * SIL:(c + 1) * SIL]
        sd_pad = np.zeros((SIP, H), BF16)
        sd_pad[:SIL] = sdT[c * SIL:(c + 1) * SIL]
        # [KSI, 128, KH*128]: [m, p, k*128+c2] = pad[k*128+p, m*128+c2]
        sg_l.append(np.ascontiguousarray(
            sg_pad.reshape(KH, 128, KSI, 128).transpose(2, 1, 0, 3)
        ).reshape(KSI, 128, KH * 128))
        su_l.append(np.ascontiguousarray(
            su_pad.reshape(KH, 128, KSI, 128).transpose(2, 1, 0, 3)
        ).reshape(KSI, 128, KH * 128))
        sd_l.append(np.ascontiguousarray(
            sd_pad.reshape(KSI, 128, H).transpose(1, 0, 2)
        ).reshape(128, KSI * H))
    packs["sg"], packs["su"], packs["sd"] = sg_l, su_l, sd_l
    _STATE["wpack"] = (key, packs)
    return packs


def kernel(**inputs) -> np.ndarray:
    inp = {k: np.ascontiguousarray(np.asarray(v), dtype=np.float32)
           for k, v in inputs.items()}
    x = inp["hidden_states"].reshape(-1, H)

    topk_idx, topk_w = _gate_host(
        x, inp["gate_weight"], inp["e_score_correction_bias"])

    # token lists per expert (ascending token order)
    idx_lists, wt_lists, counts = [], [], []
    for e in range(E):
        tok, slot = np.nonzero(topk_idx == e)
        idx_lists.append(tok)
        wt_lists.append(topk_w[tok, slot])
        counts.append(len(tok))
    counts = np.asarray(counts)

    # assign experts to (core, slot) by sorted load; slot capacity =
    # rank-group max rounded up to 2 (min 16)
    order = np.argsort(-counts, kind="stable")
    assign = np.empty((NCORES, EPC), np.int64)
    caps = []
    for s in range(EPC):
        grp = order[s * NCORES:(s + 1) * NCORES]
        assign[:, s] = grp
        caps.append(max(16, int(-(-int(counts[grp].max()) // 2) * 2)))
    caps = tuple(caps)
    CT = sum(caps)
    coff = [sum(caps[:s]) for s in range(EPC)]

    x16 = _round_bf16(x)
    xT16 = np.ascontiguousarray(x16.T)               # [H, T]
    # xs: [TT, 128, KH*512] per token-block h-chunk packing
    xs_pack = np.ascontiguousarray(
        xT16.reshape(KH, 128, TT, 512).transpose(2, 1, 0, 3)
    ).reshape(TT, 128, KH * 512)
    packs = _weight_packs(inp)

    in_maps = []
    for c in range(NCORES):
        xga = np.zeros((H, CT), BF16)
        wg_arr = np.empty((EPC, MI, 128, KH * 128), BF16)
        wu_arr = np.empty((EPC, MI, 128, KH * 128), F8E3)
        wd_arr = np.empty((EPC, MI, 2, 128, H // 2), F8E3)
        for s in range(EPC):
            e = int(assign[c, s])
            idx = idx_lists[e]
            xga[:, coff[s]:coff[s] + len(idx)] = x16[idx].T
            wg_arr[s] = packs["wg"][e]
            wu_arr[s] = packs["wu"][e]
            wd_arr[s] = packs["wd"][e]
        in_maps.append({
            "xg": _pack_hchunks(xga),
            "wg": wg_arr,
            "wu": wu_arr,
            "wd": wd_arr,
            "xs": xs_pack,
            "sg": packs["sg"][c],
            "su": packs["su"][c],
            "sd": packs["sd"][c],
        })

    nc = _get_nc(caps)
    _STATE["last_in_maps"] = in_maps
    _STATE["last_caps"] = caps
    # the accelerator very occasionally reports a transient
    # NRT_EXEC_UNIT_UNRECOVERABLE; retry a couple of times
    last_exc = None
    for _attempt in range(3):
        try:
            res = run_bass_kernel_spmd(nc, in_maps, core_ids=list(range(NCORES)))
            break
        except Exception as exc:  # noqa: BLE001
            last_exc = exc
            import time as _time
            _time.sleep(5.0)
    else:
        raise last_exc

    out = np.zeros((T, H), np.float32)
    for c in range(NCORES):
        out += res.results[c]["ys"].astype(np.float32)
    for c in range(NCORES):
        yrc = res.results[c]["yr"]                   # [H, CT] bf16, x64^2
        for s in range(EPC):
            e = int(assign[c, s])
            idx = idx_lists[e]
            if len(idx):
                w = (wt_lists[e] / (WSCALE * WSCALE)).astype(np.float32)
                out[idx] += w[:, None] * \
                    yrc[:, coff[s]:coff[s] + len(idx)].astype(np.float32).T

    return out.reshape(1, T, H).astype(np.float32)


# revision 31
# speedup vs baseline: 1.0464x; 1.0464x over previous
"""DeepseekV3 MoE layer on 8 trn2 NeuronCores (expert-parallel).

Strategy
--------
* Routing (sigmoid gate + grouped top-k) runs on host in numpy: it is
  ~0.1% of the FLOPs and it *determines* the sharding (which tokens go
  to which core), i.e. it is the dispatch step of the all-to-all.
* Experts are sharded 4-per-core, assigned by sorted load so that slot
  capacities (compile-time matmul shapes, shared by all cores under
  SPMD) can be tight: slot s on every core gets an expert from
  load-rank group s, and the slot capacity is the rank-group max.
* Gate/up expert weights are streamed as fp8 e3m4 (x64 scale, undone
  by the Silu activation's scale argument): the kernel is HBM-bound on
  weight streaming, and the PE consumes fp8 stationary x bf16 moving
  at full per-element precision (fp22 upconvert, no DoubleRow).
  w_down and the shared expert stay bf16 for error margin.
* Per core, per expert slot: gT/uT = W@xT accumulated over 16 H-chunks
  in PSUM, a = silu(g)*u evicted to SBUF as bf16 [I, C]; the down
  projection runs transposed: y[h, t] accumulated over 11 I-chunks
  with N = cap token columns (no padded 512-wide tiles), DMA'd out as
  [H, C] bf16.  The combine weight (and the 1/64 fp8 scale) is applied
  on the host during scatter-add, which is free.
* The shared expert is sharded over its intermediate dim SI (352/core,
  padded to 384): every core computes a partial [T, H] contribution.
  Its gate/up half runs FIRST (small inputs -> PE busy while the big
  routed weight stream ramps) with per-m-tile weight DMAs so the first
  matmul starts after ~3 MB; its down-projection runs LAST.  Output
  stores go through SWDGE (gpsimd) so they never block load issue on
  the SP HWDGE ring.
* Host combine: sum the 8 shared partials, weighted scatter-add of the
  32 compact expert outputs.
"""

import numpy as np
import ml_dtypes

import concourse.bass as bass
import concourse.mybir as mybir
import concourse.tile as tile
from concourse.bass_utils import run_bass_kernel_spmd

BF16 = ml_dtypes.bfloat16
F8E3 = ml_dtypes.float8_e3m4
WSCALE = 64.0

# ---- problem constants (fixed by the spec) ----
E, G, EPG, TKG, TOPK = 32, 8, 4, 4, 4
H, I, SI, SCALE = 2048, 1408, 2816, 2.5
T = 1024
NCORES = 8
EPC = E // NCORES          # experts per core = 4
KH = H // 128              # 16 contraction chunks over H
MI = I // 128              # 11 tiles over I
NH = H // 128              # 16 output tiles over H (down projection)
SIL = SI // NCORES         # 352 local shared-intermediate
SIP = 384                  # padded to 3*128
KSI = SIP // 128           # 3
HT = H // 512              # 4 output tiles over H (shared down)
TT = T // 512              # 2 tiles over tokens (shared gate/up)

_STATE: dict = {}

_TPB_ENGINES = {"Pool", "Activation", "PE", "DVE", "SP"}


def _split_multiwait_bir(bir_bytes: bytes) -> bytes:
    """Walrus codegen here accepts at most one sem-wait per TPB
    instruction.  Move excess waits onto single-wait NoOps inserted
    immediately before the instruction on the same engine (engine
    streams are in-order, and sem-ge waits are monotonic, so the chain
    is equivalent to the conjunction)."""
    import orjson

    bir = orjson.loads(bir_bytes)
    ctr = 0
    for f in bir["functions"]:
        for blk in f["blocks"]:
            out = []
            for inst in blk["instructions"]:
                si = inst.get("sync_info")
                waits = (si or {}).get("on_wait") or []
                if len(waits) > 1 and inst.get("engine") in _TPB_ENGINES:
                    for w in waits[:-1]:
                        ctr += 1
                        out.append({
                            "debug": inst.get("debug", 0),
                            "engine": inst["engine"],
                            "ins": [],
                            "outs": [],
                            "name": f"I-wsplit-{ctr}",
                            "opcode": "NoOp",
                            "sync_info": {"on_update": [], "on_wait": [w]},
                        })
                    si["on_wait"] = waits[-1:]
                out.append(inst)
            blk["instructions"] = out
    return orjson.dumps(bir)


def _patch_tile():
    if _STATE.get("patched"):
        return
    from concourse.tile import ScopedClock, TileContext

    _orig_to_json = bass.Bass.to_json_bytes

    def to_json_bytes_split(self):
        return _split_multiwait_bir(_orig_to_json(self))

    bass.Bass.to_json_bytes = to_json_bytes_split

    def _drain_and_barrier_split(self, tick_clock, wait_clock):
        probe = self.nc.sync.nop(nofuse=True)
        wait_clock.add_sem_waits(
            probe.ins, ScopedClock({None: tick_clock.global_clock})
        )
        waits = list(probe.ins.sync_info.on_wait) if probe.ins.sync_info else []
        if probe.ins.sync_info:
            probe.ins.sync_info.on_wait = waits[:1]
            for w in waits[1:]:
                n2 = self.nc.sync.nop(nofuse=True)
                si = n2.ins.sync_info
                if si is None:
                    n2.ins.sync_info = mybir.SyncInfo(on_wait=[w], on_update=[])
                else:
                    si.on_wait = [w]
        self.nc.sync.drain()
        self.nc.all_engine_barrier()
        assert self.sems is not None
        popped = self.nc._tile_sem_poison_stack.pop()
        assert popped is self._sem_poison
        self.nc.clear_and_free_semaphores(list(self.sems.allocated().values()))
        self.nc.all_engine_barrier()

    TileContext._drain_and_barrier = _drain_and_barrier_split
    _STATE["patched"] = True


def _round_bf16(a: np.ndarray) -> np.ndarray:
    """fp32 -> bf16 with round-to-nearest-even, fast pure-numpy path."""
    u = np.ascontiguousarray(a, dtype=np.float32).view(np.uint32)
    r = ((u >> 16) & 1) + np.uint32(0x7FFF)
    return ((u + r) >> np.uint32(16)).astype(np.uint16).view(BF16)


# --------------------------------------------------------------------
# host routing — exact numpy mirror of the reference gate
# --------------------------------------------------------------------
def _gate_host(x, gate_weight, bias):
    Tn = x.shape[0]
    logits = x @ gate_weight.T                       # [T, E]
    scores = 1.0 / (1.0 + np.exp(-logits))
    sfc = scores + bias[None, :]
    gs = sfc.reshape(Tn, G, EPG)
    top2 = np.sort(gs, axis=-1)[:, :, -2:].sum(-1)   # [T, G]
    grp_idx = np.argsort(-top2, axis=-1, kind="stable")[:, :TKG]
    gmask = np.zeros((Tn, G), bool)
    gmask[np.arange(Tn)[:, None], grp_idx] = True
    smask = np.repeat(gmask, EPG, axis=1)
    tmp = np.where(smask, sfc, 0.0)
    topk_idx = np.argsort(-tmp, axis=-1, kind="stable")[:, :TOPK]
    topk_w = np.take_along_axis(scores, topk_idx, axis=1)
    topk_w = topk_w / (topk_w.sum(-1, keepdims=True) + 1e-20)
    return topk_idx, topk_w * SCALE


# --------------------------------------------------------------------
# device kernel (parameterized by per-slot capacities)
# --------------------------------------------------------------------
def _build_nc(caps):
    _patch_tile()
    nc = bass.Bass("TRN2", target_bir_lowering=False, debug=False, num_devices=1)
    f32, bf, f8 = mybir.dt.float32, mybir.dt.bfloat16, mybir.dt.float8e3
    CT = sum(caps)           # total token capacity per core
    CMX = max(caps)
    coff = [sum(caps[:s]) for s in range(EPC)]  # xg column offsets

    xg = nc.dram_tensor("xg", [128, KH * CT], bf, kind="ExternalInput").ap()
    wg = nc.dram_tensor("wg", [EPC, MI, 128, KH * 128], bf, kind="ExternalInput").ap()
    wu = nc.dram_tensor("wu", [EPC, MI, 128, KH * 128], f8, kind="ExternalInput").ap()
    wd = nc.dram_tensor("wd", [EPC, MI, 128, H], f8, kind="ExternalInput").ap()
    xs = nc.dram_tensor("xs", [TT, 128, KH * 512], bf, kind="ExternalInput").ap()
    sg = nc.dram_tensor("sg", [KSI, 128, KH * 128], bf, kind="ExternalInput").ap()
    su = nc.dram_tensor("su", [KSI, 128, KH * 128], bf, kind="ExternalInput").ap()
    sd = nc.dram_tensor("sd", [128, KSI * H], bf, kind="ExternalInput").ap()
    yr = nc.dram_tensor("yr", [H, CT], bf, kind="ExternalOutput").ap()
    ys = nc.dram_tensor("ys", [T, H], bf, kind="ExternalOutput").ap()

    SILU = mybir.ActivationFunctionType.Silu

    with tile.TileContext(nc) as tc:
        with tc.tile_pool(name="main", bufs=1) as pool, \
             tc.tile_pool(name="psum", bufs=1, space="PSUM") as pp:
            # ---- startup DMAs, split across both HWDGE queues; the
            # m=0 shared tiles and the nt=0 x block are split in k-halves
            # so the first matmul starts after ~1.3 MB.  SP queue:
            # sg0/su0, xg, sd, the wu/wd fp8 streams and the tail ys
            # stores.  ACT queue: xs, sg1/2, su1/2 and the bf16 wg
            # stream.
            KH2 = KH // 2
            sg0h, su0h, xs0h = [], [], []
            for h in range(2):
                sg0h.append(pool.tile([128, KH2 * 128], bf, tag="sg0",
                                      bufs=2, name=f"sg0{h}"))
                su0h.append(pool.tile([128, KH2 * 128], bf, tag="su0",
                                      bufs=2, name=f"su0{h}"))
                xs0h.append(pool.tile([128, KH2 * 512], bf, tag="xs0",
                                      bufs=2, name=f"xs0{h}"))
            sg_sbs, su_sbs = [None], [None]
            for m in range(1, KSI):
                sg_sbs.append(pool.tile([128, KH * 128], bf, tag="sg",
                                        bufs=KSI - 1, name=f"sg{m}"))
                su_sbs.append(pool.tile([128, KH * 128], bf, tag="su",
                                        bufs=KSI - 1, name=f"su{m}"))
            xs1_sb = pool.tile([128, KH * 512], bf, tag="xs", bufs=1)
            xg_sb = pool.tile([128, KH * CT], bf, tag="xg", bufs=1)
            sd_sb = pool.tile([128, KSI * H], bf, tag="sd", bufs=1)
            # front ordering is byte-exact: both queues deliver the
            # shared-expert inputs in consumption order, then ACT carries
            # the bf16 wg stream while SP carries xg + wu/wd fp8.
            nc.scalar.dma_start(xs0h[0][:], xs[0, :, :KH2 * 512])
            nc.sync.dma_start(sg0h[0][:], sg[0, :, :KH2 * 128])
            nc.sync.dma_start(su0h[0][:], su[0, :, :KH2 * 128])
            nc.sync.dma_start(sg0h[1][:], sg[0, :, KH2 * 128:])
            nc.sync.dma_start(su0h[1][:], su[0, :, KH2 * 128:])
            nc.sync.dma_start(xs0h[1][:], xs[0, :, KH2 * 512:])
            nc.sync.dma_start(sg_sbs[1][:], sg[1])
            nc.sync.dma_start(su_sbs[1][:], su[1])
            nc.scalar.dma_start(xs1_sb[:], xs[1])
            nc.scalar.dma_start(sg_sbs[2][:], sg[2])
            nc.scalar.dma_start(su_sbs[2][:], su[2])
            nc.sync.dma_start(xg_sb[:], xg[:])

            def shared_w(ws, m, k):
                # weight lhsT [128, 128] for shared m-tile, chunk k
                if m == 0:
                    hh = sg0h if ws == "g" else su0h
                    t = hh[k // KH2]
                    kk = k % KH2
                else:
                    t = sg_sbs[m] if ws == "g" else su_sbs[m]
                    kk = k
                return t[:, kk * 128:(kk + 1) * 128]

            def shared_x(nt, k):
                if nt == 0:
                    return xs0h[k // KH2][:, (k % KH2) * 512:
                                          (k % KH2 + 1) * 512]
                return xs1_sb[:, k * 512:(k + 1) * 512]

            # ---- shared expert gate/up (sharded over SI) ----
            # nt-outer: the nt=0 groups need only xs0, deferring xs1's
            # 2.1 MB by ~20us so the front stream stays sustainable.
            as_sb = pool.tile([128, KSI * T], bf, tag="as", bufs=1)
            for nt in range(TT):
                for m in range(KSI):
                    pg = pp.tile([128, 512], f32, tag="pg", bufs=2,
                                 name=f"psg{m}_{nt}")
                    pu = pp.tile([128, 512], f32, tag="pu", bufs=2,
                                 name=f"psu{m}_{nt}")
                    if nt == 0 and m == 0:
                        # first group: interleave pg/pu k-halves so the
                        # first 16 matmuls need only the *a* half-tiles
                        krs = [(pg, "g", range(KH2)), (pu, "u", range(KH2)),
                               (pg, "g", range(KH2, KH)),
                               (pu, "u", range(KH2, KH))]
                    else:
                        krs = [(pg, "g", range(KH)), (pu, "u", range(KH))]
                    for (ps, ws, krange) in krs:
                        for k in krange:
                            nc.tensor.matmul(
                                ps[:], shared_w(ws, m, k), shared_x(nt, k),
                                start=(k == 0), stop=(k == KH - 1))
                    sil = pool.tile([128, 512], f32, tag="sil", bufs=2,
                                    name=f"ssil{m}_{nt}")
                    nc.scalar.activation(sil[:], pg[:], SILU)
                    nc.vector.tensor_mul(
                        as_sb[:, m * T + nt * 512: m * T + (nt + 1) * 512],
                        sil[:], pu[:])

            # ---- routed experts ----
            for s in range(EPC):
                cap = caps[s]
                a_sb = pool.tile([128, MI * CMX], bf, tag="a", bufs=2,
                                 name=f"a{s}")
                for m in range(MI):
                    wg_sb = pool.tile([128, KH * 128], bf, tag="wg", bufs=8,
                                      name=f"wg{s}_{m}")
                    nc.scalar.dma_start(wg_sb[:], wg[s, m])
                    wu_sb = pool.tile([128, KH * 128], f8, tag="wu", bufs=8,
                                      name=f"wu{s}_{m}")
                    nc.sync.dma_start(wu_sb[:], wu[s, m])
                    pg = pp.tile([128, cap], f32, tag="pg", bufs=2,
                                 padded_shape=[128, 512], name=f"pg{s}_{m}")
                    pu = pp.tile([128, cap], f32, tag="pu", bufs=2,
                                 padded_shape=[128, 512], name=f"pu{s}_{m}")
                    for k in range(KH):
                        nc.tensor.matmul(
                            pg[:], wg_sb[:, k * 128:(k + 1) * 128],
                            xg_sb[:, k * CT + coff[s]: k * CT + coff[s] + cap],
                            start=(k == 0), stop=(k == KH - 1))
                    for k in range(KH):
                        nc.tensor.matmul(
                            pu[:], wu_sb[:, k * 128:(k + 1) * 128],
                            xg_sb[:, k * CT + coff[s]: k * CT + coff[s] + cap],
                            start=(k == 0), stop=(k == KH - 1))
                    sil = pool.tile([128, cap], f32, tag="sil", bufs=2,
                                    padded_shape=[128, 512], name=f"sil{s}_{m}")
                    nc.scalar.activation(sil[:], pg[:], SILU)
                    nc.vector.tensor_mul(
                        a_sb[:, m * cap:(m + 1) * cap], sil[:], pu[:])

                wd_sbs = []
                for k2 in range(MI):
                    wdt = pool.tile([128, H], f8, tag="wd", bufs=13,
                                    name=f"wd{s}_{k2}")
                    nc.sync.dma_start(wdt[:], wd[s, k2])
                    wd_sbs.append(wdt)
                if s == 0:
                    # shared down-proj weights: needed only at the end —
                    # stream them behind the first expert's.
                    nc.sync.dma_start(sd_sb[:], sd[:])

                if s == EPC - 1:
                    # ---- shared expert down-projection, emitted between
                    # the last slot's gate/up and down phases: its ys
                    # stores are issued after the wd DMAs on the SP queue
                    # (no head-of-line block of loads) and drain while
                    # the last down phase computes, instead of as an
                    # exposed tail.
                    for mt in range(T // 128):
                        for n in range(HT):
                            py = pp.tile([128, 512], f32, tag="py", bufs=4,
                                         name=f"pys{mt}_{n}")
                            for k in range(KSI):
                                nc.tensor.matmul(
                                    py[:],
                                    as_sb[:, k * T + mt * 128:
                                          k * T + (mt + 1) * 128],
                                    sd_sb[:, k * H + n * 512:
                                          k * H + (n + 1) * 512],
                                    start=(k == 0), stop=(k == KSI - 1))
                            yo = pool.tile([128, 512], bf, tag="yo", bufs=8,
                                           name=f"yos{mt}_{n}")
                            nc.vector.tensor_copy(yo[:], py[:])
                            nc.sync.dma_start(
                                ys[mt * 128:(mt + 1) * 128,
                                   n * 512:(n + 1) * 512],
                                yo[:])

                # transposed down-projection: y[h, t], N = cap columns,
                # contraction over I.  No padded token tiles.
                for n in range(NH):
                    py = pp.tile([128, cap], f32, tag="py", bufs=4,
                                 padded_shape=[128, 512], name=f"py{s}_{n}")
                    for k2 in range(MI):
                        nc.tensor.matmul(
                            py[:],
                            wd_sbs[k2][:, n * 128:(n + 1) * 128],
                            a_sb[:, k2 * cap: k2 * cap + cap],
                            start=(k2 == 0), stop=(k2 == MI - 1))
                    yo = pool.tile([128, cap], bf, tag="yo", bufs=8,
                                   padded_shape=[128, 512], name=f"yo{s}_{n}")
                    nc.vector.tensor_copy(yo[:], py[:])
                    if s == EPC - 1:
                        # last slot's stores drain through the ACT HWDGE
                        # queue (its wg load stream is done by now) so the
                        # kernel tail is not paced by SWDGE.
                        nc.scalar.dma_start(
                            yr[n * 128:(n + 1) * 128,
                               coff[s]: coff[s] + cap], yo[:])
                    else:
                        nc.gpsimd.dma_start(
                            yr[n * 128:(n + 1) * 128,
                               coff[s]: coff[s] + cap], yo[:])


    return nc


def _get_nc(caps):
    key = ("nc", tuple(caps))
    if key not in _STATE:
        _STATE[key] = _build_nc(caps)
    return _STATE[key]


# --------------------------------------------------------------------
# host packing
# --------------------------------------------------------------------
def _pack_weight_gate_up(w_e, dt, scale):
    # w_e: [I, H] f32 -> dt x scale, packed [MI, 128, KH*128]
    # with [m, p, k*128+c] = w[m*128+c, k*128+p]
    wq = (w_e * scale).astype(dt) if dt is F8E3 else _round_bf16(w_e)
    return np.ascontiguousarray(
        wq.reshape(MI, 128, KH, 128).transpose(0, 3, 2, 1)
    ).reshape(MI, 128, KH * 128)


def _pack_weight_down(w_e):
    # w_e: [H, I] f32 -> fp8 e3m4 x WSCALE, [MI, 128, H] with
    # [k2, p, h] = w[h, k2*128+p]
    w8 = (w_e * WSCALE).astype(F8E3)
    return np.ascontiguousarray(
        w8.reshape(H, MI, 128).transpose(1, 2, 0))


def _pack_hchunks(a16):
    # a16: [H, N] bf16 -> [128, KH*N] with [p, k*N+j] = a[k*128+p, j]
    N = a16.shape[1]
    return np.ascontiguousarray(
        a16.reshape(KH, 128, N).transpose(1, 0, 2)).reshape(128, KH * N)


def _weight_packs(inp):
    """Pack (and cache) the routed + shared weights; they do not depend
    on routing, only on the weight tensors themselves."""
    key = tuple(inp[k].ctypes.data for k in
                ("w_gate", "w_up", "w_down", "shared_w_gate",
                 "shared_w_up", "shared_w_down"))
    cached = _STATE.get("wpack")
    if cached is not None and cached[0] == key:
        return cached[1]

    packs = {
        "wg": [_pack_weight_gate_up(inp["w_gate"][e], BF16, 1.0)
               for e in range(E)],
        "wu": [_pack_weight_gate_up(inp["w_up"][e], F8E3, WSCALE)
               for e in range(E)],
        "wd": [_pack_weight_down(inp["w_down"][e]) for e in range(E)],
    }
    sgT = _round_bf16(inp["shared_w_gate"]).T        # [H, SI]
    suT = _round_bf16(inp["shared_w_up"]).T
    sdT = _round_bf16(inp["shared_w_down"]).T        # [SI, H]
    sg_l, su_l, sd_l = [], [], []
    for c in range(NCORES):
        sg_pad = np.zeros((H, SIP), BF16)
        sg_pad[:, :SIL] = sgT[:, c * SIL:(c + 1) * SIL]
        su_pad = np.zeros((H, SIP), BF16)
        su_pad[:, :SIL] = suT[:, c * SIL:(c + 1) * SIL]
        sd_pad = np.zeros((SIP, H), BF16)
        sd_pad[:SIL] = sdT[c * SIL:(c + 1) * SIL]
        # [KSI, 128, KH*128]: [m, p, k*128+c2] = pad[k*128+p, m*128+c2]
        sg_l.append(np.ascontiguousarray(
            sg_pad.reshape(KH, 128, KSI, 128).transpose(2, 1, 0, 3)
        ).reshape(KSI, 128, KH * 128))
        su_l.append(np.ascontiguousarray(
            su_pad.reshape(KH, 128, KSI, 128).transpose(2, 1, 0, 3)
        ).reshape(KSI, 128, KH * 128))
        sd_l.append(np.ascontiguousarray(
            sd_pad.reshape(KSI, 128, H).transpose(1, 0, 2)
        ).reshape(128, KSI * H))
    packs["sg"], packs["su"], packs["sd"] = sg_l, su_l, sd_l
    _STATE["wpack"] = (key, packs)
    return packs


def kernel(**inputs) -> np.ndarray:
    inp = {k: np.ascontiguousarray(np.asarray(v), dtype=np.float32)
           for k, v in inputs.items()}
    x = inp["hidden_states"].reshape(-1, H)

    topk_idx, topk_w = _gate_host(
        x, inp["gate_weight"], inp["e_score_correction_bias"])

    # token lists per expert (ascending token order)
    idx_lists, wt_lists, counts = [], [], []
    for e in range(E):
        tok, slot = np.nonzero(topk_idx == e)
        idx_lists.append(tok)
        wt_lists.append(topk_w[tok, slot])
        counts.append(len(tok))
    counts = np.asarray(counts)

    # assign experts to (core, slot) by sorted load; slot capacity =
    # rank-group max rounded up to 2 (min 16)
    order = np.argsort(-counts, kind="stable")
    assign = np.empty((NCORES, EPC), np.int64)
    caps = []
    for s in range(EPC):
        grp = order[s * NCORES:(s + 1) * NCORES]
        assign[:, s] = grp
        caps.append(max(16, int(-(-int(counts[grp].max()) // 2) * 2)))
    caps = tuple(caps)
    CT = sum(caps)
    coff = [sum(caps[:s]) for s in range(EPC)]

    x16 = _round_bf16(x)
    xT16 = np.ascontiguousarray(x16.T)               # [H, T]
    # xs: [TT, 128, KH*512] per token-block h-chunk packing
    xs_pack = np.ascontiguousarray(
        xT16.reshape(KH, 128, TT, 512).transpose(2, 1, 0, 3)
    ).reshape(TT, 128, KH * 512)
    packs = _weight_packs(inp)

    in_maps = []
    for c in range(NCORES):
        xga = np.zeros((H, CT), BF16)
        wg_arr = np.empty((EPC, MI, 128, KH * 128), BF16)
        wu_arr = np.empty((EPC, MI, 128, KH * 128), F8E3)
        wd_arr = np.empty((EPC, MI, 128, H), F8E3)
        for s in range(EPC):
            e = int(assign[c, s])
            idx = idx_lists[e]
            xga[:, coff[s]:coff[s] + len(idx)] = x16[idx].T
            wg_arr[s] = packs["wg"][e]
            wu_arr[s] = packs["wu"][e]
            wd_arr[s] = packs["wd"][e]
        in_maps.append({
            "xg": _pack_hchunks(xga),
            "wg": wg_arr,
            "wu": wu_arr,
            "wd": wd_arr,
            "xs": xs_pack,
            "sg": packs["sg"][c],
            "su": packs["su"][c],
            "sd": packs["sd"][c],
        })

    nc = _get_nc(caps)
    _STATE["last_in_maps"] = in_maps
    _STATE["last_caps"] = caps
    # the accelerator very occasionally reports a transient
    # NRT_EXEC_UNIT_UNRECOVERABLE; retry a couple of times
    last_exc = None
    for _attempt in range(3):
        try:
            res = run_bass_kernel_spmd(nc, in_maps, core_ids=list(range(NCORES)))
            break
        except Exception as exc:  # noqa: BLE001
            last_exc = exc
            import time as _time
            _time.sleep(5.0)
    else:
        raise last_exc

    out = np.zeros((T, H), np.float32)
    for c in range(NCORES):
        out += res.results[c]["ys"].astype(np.float32)
    for c in range(NCORES):
        yrc = res.results[c]["yr"]                   # [H, CT] bf16, x64^2
        for s in range(EPC):
            e = int(assign[c, s])
            idx = idx_lists[e]
            if len(idx):
                w = (wt_lists[e] / (WSCALE * WSCALE)).astype(np.float32)
                out[idx] += w[:, None] * \
                    yrc[:, coff[s]:coff[s] + len(idx)].astype(np.float32).T

    return out.reshape(1, T, H).astype(np.float32)
